# revision 1
# baseline (speedup 1.0000x reference)
"""Trainium2 Bass kernel: single-head GATConv (+ self-loops, segment softmax)
followed by LayerNorm, distributed over 8 NeuronCores.

Distribution (destination-sharded SPMD):
  * Host: append self-loops, sort edges by destination, shard destinations
    contiguously across cores.  Within a core, destinations form 128-wide
    blocks.  Each block's edges are split into 8 subgroups:
    (sign of a_s[src]+a_d[dst]) x (src bank of 25088 rows, for int16
    dma_gather indices), each padded to a multiple of 128 slots.  Subgroup
    widths are uniform across blocks/cores so one program serves all cores.
  * Softmax factorization: exp(leaky_relu(s+d)) = max(us*ud, vs*vd) with
    us=exp(s), vs=exp(0.2 s) (per-source), ud=exp(d), vd=exp(0.2 d)
    (per-dest).  The host decides each edge's max-branch by sign (control
    only); the device then needs NO per-edge transcendentals: the dest
    factors are uniform per matmul column group and pull out of the
    aggregation matmul.
  * Device phase A (replicated): hext[n] = bf16 row
    [h(0:64) | 1 | a_s_hi | a_d_hi | a_s_lo | a_d_lo | pad] (128 cols =
    256 B, dma_gather's minimum row).  hi/lo bf16 splits keep exp args
    accurate to ~1e-4.
  * Device phase B: per chunk of CB blocks: 8 dma_gather calls fetch
    hext[src] rows into G; U = exp(a_s_hi)*exp(a_s_lo), V = same with 0.2
    scale (batched ACT); per column: one fused DVE op builds
    A^T = (iota == dst_rel) * U_col, one matmul accumulates into the
    pos/neg halves of the block's PSUM acc; per-block epilogue applies
    ud/vd column scales, (denominator+bias when needed) and LayerNorm.
"""

import math

import numpy as np
import ml_dtypes

import concourse.bacc as bacc
import concourse.bass as bass
import concourse.tile as tile
from concourse import mybir
from concourse.bass_utils import run_bass_kernel_spmd

P = 128
HEXT_W = 128          # f32 row = 512 B (dma_gather needs 256B-multiple rows)
COL_ONES = 64
COL_AS = 65
COL_AD = 66
N_BANKS = 4           # dma_gather int16 indices: banks of N_pad//4 <= 32767
N_GROUPS = 2          # positive / negative leaky branch

f32 = mybir.dt.float32
bf16 = mybir.dt.bfloat16
i16 = mybir.dt.int16

LEAK = 0.2
LN_EPS = 1e-5

bfdt = ml_dtypes.bfloat16


def _cdiv(a, b):
    return -(-a // b)


# ---------------------------------------------------------------------------
# Host-side preprocessing
# ---------------------------------------------------------------------------

def prep_edges(x, edge_index, W, att_src, att_dst, n_cores, bank_size):
    """Shard + sort edges, split into (sign x bank) subgroups per 128-dest
    block, pad subgroups to x128 slots with uniform widths.

    Returns (per_core, S_gk, NB, nd_core):
      per_core[c] = dict(idx=..., dr=...) where
        idx: int16 [P, total_idx_cols]  packed dma_gather indices, call-major
        dr:  bf16  [P, TC]              per-slot dest-slot-in-block (-1 pad)
      S_gk: [8] list of per-subgroup column counts (order: g-major, k-minor)
    """
    N, D = x.shape
    assert N % n_cores == 0
    nd_core = N // n_cores
    NB = _cdiv(nd_core, P)

    ws = (W @ att_src).astype(np.float64)
    wd = (W @ att_dst).astype(np.float64)
    a_s = (x.astype(np.float64) @ ws).astype(np.float32)
    a_d = (x.astype(np.float64) @ wd).astype(np.float32)

    src = np.asarray(edge_index[0]).astype(np.int64)
    dst = np.asarray(edge_index[1]).astype(np.int64)
    loops = np.arange(N, dtype=np.int64)
    src = np.concatenate([src, loops])
    dst = np.concatenate([dst, loops])

    # edge keys: (dest block within core handled later); global sort by dst
    order = np.argsort(dst, kind="stable")
    s_dst = dst[order]
    s_src = src[order]
    sign = (a_s[s_src] + a_d[s_dst]) <= 0.0   # False=pos group0, True=neg

    bounds = np.searchsorted(s_dst, np.arange(0, N + nd_core, nd_core))

    # per (core, block, group, bank) counts.  Each core's hext uses node
    # order rotated by c*nd_core so its own destinations sit at rows
    # [0, nd_core) — the SPMD program then loads dest rows at fixed
    # addresses on every core.
    cnts = np.zeros((n_cores, NB, N_GROUPS, N_BANKS), dtype=np.int64)
    per_core_raw = []
    for c in range(n_cores):
        lo, hi = int(bounds[c]), int(bounds[c + 1])
        d_loc = s_dst[lo:hi] - c * nd_core
        blk = d_loc >> 7
        g = sign[lo:hi].astype(np.int64)
        src_row = (s_src[lo:hi] - c * nd_core) % N   # rotated hext row
        k = src_row // bank_size
        key = ((blk * N_GROUPS + g) * N_BANKS + k)
        cnts[c] += np.bincount(key, minlength=NB * 8).reshape(NB, 2, 4)
        per_core_raw.append((d_loc, src_row, blk, g, k, key))

    S_gk = [int(_cdiv(int(cnts[:, :, g, k].max()), P))
            for g in range(N_GROUPS) for k in range(N_BANKS)]
    TC_BLK = sum(S_gk)

    # column offsets: per chunk columns are laid out call-major:
    # [ (g,k)=0 cols of all blocks | (g,k)=1 ... ]  -- decided at chunk
    # granularity in the device program; here we produce per-slot arrays in
    # GLOBAL column order  col(b, gk, s) = b*TC_BLK + off_gk + s  and the
    # device/chunk mapping rearranges.  To keep host/device agreement simple
    # we instead directly emit per-slot data in *block-major* order and let
    # the idx slabs be built per (chunk, call) below.
    off_gk = np.concatenate([[0], np.cumsum(S_gk)])[:-1]

    per_core = []
    for c in range(n_cores):
        d_loc, src_c, blk, g, k, key = per_core_raw[c]
        # slot within subgroup
        order2 = np.argsort(key, kind="stable")
        d2 = d_loc[order2]
        s2 = src_c[order2]
        key2 = key[order2]
        starts = np.zeros(NB * 8 + 1, dtype=np.int64)
        starts[1:] = np.cumsum(np.bincount(key2, minlength=NB * 8))
        pos_in = np.arange(len(key2)) - starts[key2]
        b2 = key2 // 8
        gk2 = key2 % 8
        # global slot id = ((b*TC_BLK + off_gk + s)*128 + lane)
        s_col = pos_in >> 7
        lane = pos_in & 127
        col = b2 * TC_BLK + off_gk[gk2] + s_col
        TC = NB * TC_BLK
        dr = np.full((P, TC), -1.0, dtype=np.float32)
        src_slot = np.zeros((P, TC), dtype=np.int64)
        dr[lane, col] = (d2 & 127).astype(np.float32)
        src_slot[lane, col] = s2 % bank_size  # bank-local row (pads stay 0)
        per_core.append({"dr": dr, "src_slot": src_slot})
    return per_core, S_gk, NB, nd_core, TC_BLK


def build_idx_slabs(per_core, S_gk, NB, CB, TC_BLK):
    """Per-core int16 idx slab [P, n_chunks * chunk_idx_cols], where each
    chunk's region is the concatenation of its 8 calls' packed index arrays.
    Call (gk) covers columns off_gk..off_gk+CB*S_gk-1 (block-major within
    call) of the chunk; idx i of a call = slot (i%128, col i//128), packed
    int16 at [16*(core) wrap]: position i -> partition i%16, word i//16,
    replicated across the 8 gpsimd core groups."""
    n_chunks = NB // CB
    off_gk = np.concatenate([[0], np.cumsum(S_gk)])[:-1]
    out = []
    for data in per_core:
        src_slot = data["src_slot"]  # [P, NB*TC_BLK] bank-local rows
        slabs = []
        for ch in range(n_chunks):
            for gk in range(8):
                sgk = S_gk[gk]
                # columns of this call: blocks ch*CB..ch*CB+CB-1
                cols = []
                for b in range(ch * CB, (ch + 1) * CB):
                    c0 = b * TC_BLK + off_gk[gk]
                    cols.append(src_slot[:, c0:c0 + sgk])
                call = np.concatenate(cols, axis=1)  # [P, CB*sgk]
                n = call.shape[1] * P
                # flat order: position i = (col i//128, lane i%128)
                flat = call.T.reshape(-1)  # [(cols, lanes)] -> i = col*128+lane
                packed = np.zeros((16, n // 16), dtype=np.int16)
                packed[np.arange(n) % 16, np.arange(n) // 16] = (
                    flat.astype(np.uint16).view(np.int16))
                slabs.append(np.tile(packed, (8, 1)))
        out.append(np.concatenate(slabs, axis=1))
    return out


def build_dr_slab(per_core, S_gk, NB, CB, TC_BLK):
    """Rearrange dr into per-chunk call-major column order used on device:
    chunk columns = [gk0: blocks b0..b0+CB x S_0 | gk1: ... ]."""
    n_chunks = NB // CB
    off_gk = np.concatenate([[0], np.cumsum(S_gk)])[:-1]
    out = []
    for data in per_core:
        dr = data["dr"]
        pieces = []
        for ch in range(n_chunks):
            for gk in range(8):
                sgk = S_gk[gk]
                for b in range(ch * CB, (ch + 1) * CB):
                    c0 = b * TC_BLK + off_gk[gk]
                    pieces.append(dr[:, c0:c0 + sgk])
        out.append(np.ascontiguousarray(np.concatenate(pieces, axis=1)))
    return out


def prep_xT(x, sup):
    """Pad x, permute within super-tiles so contiguous lhsT slices produce
    per-partition-contiguous hext stores, return [D, N_pad] f32."""
    N, D = x.shape
    n_sup = _cdiv(N, sup)
    N_pad = n_sup * sup
    xpad = np.zeros((N_pad, D), dtype=np.float32)
    xpad[:N] = x
    MP = sup // P
    xr = xpad.reshape(n_sup, P, MP, D)
    xperm = xr.transpose(0, 2, 1, 3).reshape(N_pad, D)
    return np.ascontiguousarray(xperm.T), N_pad


def make_w_aug(W, att_src, att_dst):
    D = W.shape[0]
    w_aug = np.zeros((D, 66), dtype=np.float32)
    w_aug[:, :D] = W
    w_aug[:, 64] = W @ att_src
    w_aug[:, 65] = W @ att_dst
    return w_aug


# ---------------------------------------------------------------------------
# Device program
# ---------------------------------------------------------------------------

def build_program(N_pad, D, NB, S_gk, CB, SUP, w_aug, general,
                  ln_bias=None, ln_gamma=None, ln_beta=None,
                  debug_stage=None, n_queues=4):
    assert NB % CB == 0
    assert SUP % P == 0 and N_pad % SUP == 0
    assert N_pad % N_BANKS == 0
    bank_size = N_pad // N_BANKS
    assert bank_size <= 32768
    MP = SUP // P
    n_sup = N_pad // SUP
    n_chunks = NB // CB
    TC_BLK = sum(S_gk)
    CS = CB * TC_BLK                  # columns per chunk
    NCOL = 65 if general else 64      # matmul rhs width
    ACC_W = 2 * NCOL                  # pos half | neg half
    off_gk = [0]
    for s in S_gk[:-1]:
        off_gk.append(off_gk[-1] + s)
    IDX_COLS = CS * 8                 # int16 words per chunk ( CS*128/16 )

    nc = bacc.Bacc(num_swdge_queues=n_queues)
    xT_d = nc.declare_dram_parameter("xT", [D, N_pad], f32, isOutput=False)
    idx_d = nc.declare_dram_parameter(
        "idx", [P, n_chunks * IDX_COLS], i16, isOutput=False)
    dr_d = nc.declare_dram_parameter(
        "dr", [P, n_chunks * CS], f32, isOutput=False)
    out_d = nc.declare_dram_parameter("out", [NB * P, D], f32, isOutput=True)
    hext = nc.dram_tensor("hext", [N_pad, HEXT_W], f32)

    w_aug_t = nc.inline_tensor(w_aug.astype(np.float32), "w_aug")
    iota_np = np.broadcast_to(
        np.arange(P, dtype=np.float32), (P, P)).copy()
    iota_t = nc.inline_tensor(iota_np, "iota_rows")
    if general:
        def _rep(v):
            return np.ascontiguousarray(np.broadcast_to(
                np.asarray(v, dtype=np.float32).reshape(1, D), (P, D)))
        bias_t = nc.inline_tensor(_rep(ln_bias), "ln_bias")
        gamma_t = nc.inline_tensor(_rep(ln_gamma), "ln_gamma")
        beta_t = nc.inline_tensor(_rep(ln_beta), "ln_beta")

    core_base_rows = NB * P  # dest rows per core handled by out indexing

    with tile.TileContext(nc) as tc:
        with tc.tile_pool(name="const", bufs=1) as cpool:
            iota_sb = cpool.tile([P, P], f32, tag="c_iota")
            nc.sync.dma_start(out=iota_sb[:], in_=iota_t[:])
            waug_sb = cpool.tile([D, 66], f32, tag="c_waug")
            nc.sync.dma_start(out=waug_sb[:], in_=w_aug_t[:])
            eps_sb = cpool.tile([P, 1], f32, tag="c_eps")
            nc.vector.memset(eps_sb[:], LN_EPS)
            if general:
                bias_sb = cpool.tile([P, D], f32, tag="c_bias")
                nc.sync.dma_start(out=bias_sb[:], in_=bias_t[:])
                gamma_sb = cpool.tile([P, D], f32, tag="c_gamma")
                nc.sync.dma_start(out=gamma_sb[:], in_=gamma_t[:])
                beta_sb = cpool.tile([P, D], f32, tag="c_beta")
                nc.sync.dma_start(out=beta_sb[:], in_=beta_t[:])

            # ---------------- Phase A ------------------------------------
            with tc.tile_pool(name="pa_x", bufs=3) as pa_x, \
                 tc.tile_pool(name="pa_ps", bufs=4, space="PSUM") as pa_ps, \
                 tc.tile_pool(name="pa_h", bufs=3) as pa_h:
                for gsup in range(n_sup):
                    xt_sb = pa_x.tile([D, SUP], f32)
                    nc.sync.dma_start(
                        out=xt_sb[:], in_=xT_d[:, gsup * SUP:(gsup + 1) * SUP])
                    hx_sb = pa_h.tile([P, MP, HEXT_W], f32)
                    for q in range(MP // 4):
                        ps = pa_ps.tile([P, 4, 66], f32)
                        for kk in range(4):
                            m = q * 4 + kk
                            nc.tensor.matmul(
                                ps[:, kk, :],
                                lhsT=xt_sb[:, m * P:(m + 1) * P],
                                rhs=waug_sb[:],
                                start=True, stop=True,
                            )
                        sl = slice(q * 4, (q + 1) * 4)
                        if q % 2 == 0:
                            nc.vector.tensor_copy(
                                out=hx_sb[:, sl, 0:64], in_=ps[:, :, 0:64])
                        else:
                            nc.scalar.copy(
                                out=hx_sb[:, sl, 0:64], in_=ps[:, :, 0:64])
                        nc.vector.tensor_copy(
                            out=hx_sb[:, sl, COL_AS:COL_AD + 1],
                            in_=ps[:, :, 64:66])
                    nc.vector.memset(hx_sb[:, :, COL_ONES:COL_ONES + 1], 1.0)
                    nc.vector.memset(hx_sb[:, :, COL_AD + 1:], 0.0)
                    nc.sync.dma_start(
                        out=hext[gsup * SUP:(gsup + 1) * SUP, :].rearrange(
                            "(p m) c -> p m c", m=MP),
                        in_=hx_sb[:],
                    )

            # ---------------- Phase B ------------------------------------
            with tc.tile_pool(name="pb_io", bufs=2) as pb_io, \
                 tc.tile_pool(name="pb_g", bufs=2) as pb_g, \
                 tc.tile_pool(name="pb_uv", bufs=2) as pb_uv, \
                 tc.tile_pool(name="pb_at", bufs=4) as pb_at, \
                 tc.tile_pool(name="pb_y", bufs=3) as pb_y, \
                 tc.tile_pool(name="pb_sm", bufs=6) as pb_sm, \
                 tc.tile_pool(name="pb_ps", bufs=4, space="PSUM") as pb_ps:
                for ch in range(n_chunks if debug_stage != "A" else 0):
                    idx_sb = pb_io.tile([P, IDX_COLS], i16)
                    nc.sync.dma_start(
                        out=idx_sb[:],
                        in_=idx_d[:, ch * IDX_COLS:(ch + 1) * IDX_COLS])
                    dr_sb = pb_io.tile([P, CS], f32)
                    nc.sync.dma_start(
                        out=dr_sb[:], in_=dr_d[:, ch * CS:(ch + 1) * CS])
                    hb_sb = pb_io.tile([P, CB, HEXT_W], f32)
                    nc.sync.dma_start(
                        out=hb_sb[:],
                        in_=hext[ch * CB * P:(ch + 1) * CB * P, :].rearrange(
                            "(b p) c -> p b c", p=P))

                    G = pb_g.tile([P, CS, HEXT_W], f32)
                    iw = 0  # idx word offset within the chunk slab
                    qn = 0
                    for gk in range(8):
                        sgk = S_gk[gk]
                        ncols = CB * sgk
                        kbank = gk % N_BANKS
                        # descriptor ring caps one call at ~1024 rows
                        for c0 in range(0, ncols, 8):
                            cols = min(8, ncols - c0)
                            nidx = cols * P
                            nc.gpsimd.dma_gather(
                                out_ap=G[:, CB * off_gk[gk] + c0:
                                         CB * off_gk[gk] + c0 + cols, :],
                                in_ap=hext[kbank * bank_size:
                                           (kbank + 1) * bank_size, :],
                                idxs_ap=idx_sb[:, iw:iw + nidx // 16],
                                num_idxs=nidx, num_idxs_reg=nidx,
                                elem_size=HEXT_W,
                                queue_num=qn)
                            qn = (qn + 1) % n_queues
                            iw += nidx // 16
                    if debug_stage == "gather":
                        continue

                    # U/V source factors (hi/lo exp product), batched
                    u_t = pb_uv.tile([P, CS], f32)
                    v_t = pb_uv.tile([P, CS], f32)
                    nc.scalar.activation(
                        out=u_t[:], in_=G[:, :, COL_AS],
                        func=mybir.ActivationFunctionType.Exp)
                    nc.scalar.activation(
                        out=v_t[:], in_=G[:, :, COL_AS],
                        func=mybir.ActivationFunctionType.Exp, scale=LEAK)

                    # dest factors ud/vd per block (hi/lo exp product)
                    ud_t = pb_uv.tile([P, CB], f32)
                    vd_t = pb_uv.tile([P, CB], f32)
                    nc.scalar.activation(
                        out=ud_t[:], in_=hb_sb[:, :, COL_AD],
                        func=mybir.ActivationFunctionType.Exp)
                    nc.scalar.activation(
                        out=vd_t[:], in_=hb_sb[:, :, COL_AD],
                        func=mybir.ActivationFunctionType.Exp, scale=LEAK)
                    if debug_stage == "uv":
                        continue

                    for bb in range(CB):
                        b = ch * CB + bb
                        acc = pb_ps.tile([P, ACC_W], f32)
                        for g in range(N_GROUPS):
                            scal = u_t if g == 0 else v_t
                            half = slice(0, NCOL) if g == 0 else \
                                slice(NCOL, 2 * NCOL)
                            colss = []
                            for k in range(N_BANKS):
                                gk = g * N_BANKS + k
                                c0 = CB * off_gk[gk] + bb * S_gk[gk]
                                colss.extend(range(c0, c0 + S_gk[gk]))
                            for ii, cc in enumerate(colss):
                                at = pb_at.tile([P, P], f32)
                                nc.vector.tensor_scalar(
                                    out=at[:], in0=iota_sb[:],
                                    scalar1=dr_sb[:, cc:cc + 1],
                                    scalar2=scal[:, cc:cc + 1],
                                    op0=mybir.AluOpType.is_equal,
                                    op1=mybir.AluOpType.mult,
                                )
                                nc.tensor.matmul(
                                    acc[:, half], lhsT=at[:],
                                    rhs=G[:, cc, 0:NCOL],
                                    start=(ii == 0),
                                    stop=(ii == len(colss) - 1),
                                )
                        # epilogue
                        t1 = pb_y.tile([P, D], f32)
                        nc.scalar.activation(
                            out=t1[:], in_=acc[:, 0:D],
                            func=mybir.ActivationFunctionType.Copy,
                            scale=ud_t[:, bb:bb + 1])
                        t2 = pb_y.tile([P, D], f32)
                        nc.scalar.activation(
                            out=t2[:], in_=acc[:, NCOL:NCOL + D],
                            func=mybir.ActivationFunctionType.Copy,
                            scale=vd_t[:, bb:bb + 1])
                        y0 = pb_y.tile([P, D], f32)
                        nc.vector.tensor_add(out=y0[:], in0=t1[:], in1=t2[:])
                        if general:
                            den = pb_sm.tile([P, 1], f32)
                            nc.scalar.activation(
                                out=den[:], in_=acc[:, 64:65],
                                func=mybir.ActivationFunctionType.Copy,
                                scale=ud_t[:, bb:bb + 1])
                            den2 = pb_sm.tile([P, 1], f32)
                            nc.scalar.activation(
                                out=den2[:], in_=acc[:, NCOL + 64:NCOL + 65],
                                func=mybir.ActivationFunctionType.Copy,
                                scale=vd_t[:, bb:bb + 1])
                            nc.vector.tensor_add(
                                out=den[:], in0=den[:], in1=den2[:])
                            rec = pb_sm.tile([P, 1], f32)
                            nc.vector.reciprocal(rec[:], den[:])
                            nc.vector.tensor_scalar_mul(
                                out=y0[:], in0=y0[:], scalar1=rec[:])
                            nc.vector.tensor_add(
                                out=y0[:], in0=y0[:], in1=bias_sb[:])
                        st = pb_sm.tile([P, 6], f32)
                        nc.vector.bn_stats(out=st[:], in_=y0[:])
                        mv = pb_sm.tile([P, 2], f32)
                        nc.vector.bn_aggr(out=mv[:], in_=st[:])
                        sd = pb_sm.tile([P, 1], f32)
                        nc.scalar.activation(
                            out=sd[:], in_=mv[:, 1:2],
                            func=mybir.ActivationFunctionType.Sqrt,
                            bias=eps_sb[:])
                        nc.vector.reciprocal(sd[:], sd[:])
                        y = pb_y.tile([P, D], f32)
                        nc.vector.tensor_scalar(
                            out=y[:], in0=y0[:],
                            scalar1=mv[:, 0:1], scalar2=sd[:],
                            op0=mybir.AluOpType.subtract,
                            op1=mybir.AluOpType.mult,
                        )
                        if general:
                            nc.vector.tensor_mul(
                                out=y[:], in0=y[:], in1=gamma_sb[:])
                            nc.vector.tensor_add(
                                out=y[:], in0=y[:], in1=beta_sb[:])
                        nc.sync.dma_start(
                            out=out_d[b * P:(b + 1) * P, :], in_=y[:])
    nc.finalize()
    return nc


# ---------------------------------------------------------------------------
# Entry point
# ---------------------------------------------------------------------------

N_CORES = 8
SUP_DEFAULT = 2048

LAST_RESULTS = None


def _pick_cb(NB):
    for cb in (7, 8, 6, 5, 4, 2):
        if NB % cb == 0:
            return cb
    return 1


def kernel(x, edge_index, W, att_src, att_dst, bias, gamma, beta):
    global LAST_RESULTS
    x = np.asarray(x, dtype=np.float32)
    W = np.asarray(W, dtype=np.float32)
    att_src = np.asarray(att_src, dtype=np.float32)
    att_dst = np.asarray(att_dst, dtype=np.float32)
    bias = np.asarray(bias, dtype=np.float32)
    gamma = np.asarray(gamma, dtype=np.float32)
    beta = np.asarray(beta, dtype=np.float32)
    N, D = x.shape

    _, N_pad = prep_xT(x, SUP_DEFAULT)
    bank_size = N_pad // N_BANKS
    nd_core = x.shape[0] // N_CORES
    xTs = [prep_xT(np.roll(x, -c * nd_core, axis=0), SUP_DEFAULT)[0]
           for c in range(N_CORES)]
    per_core, S_gk, NB, nd_core, TC_BLK = prep_edges(
        x, edge_index, W, att_src, att_dst, N_CORES, bank_size)
    CB = _pick_cb(NB)
    idx_slabs = build_idx_slabs(per_core, S_gk, NB, CB, TC_BLK)
    dr_slabs = build_dr_slab(per_core, S_gk, NB, CB, TC_BLK)
    w_aug = make_w_aug(W, att_src, att_dst)
    general = not (
        np.all(bias == 0.0) and np.all(gamma == 1.0) and np.all(beta == 0.0))

    nc = build_program(
        N_pad, D, NB, S_gk, CB, SUP_DEFAULT, w_aug, general,
        ln_bias=bias, ln_gamma=gamma, ln_beta=beta)

    in_maps = []
    for c in range(N_CORES):
        in_maps.append(
            {"xT": xTs[c], "idx": idx_slabs[c], "dr": dr_slabs[c]})

    res = run_bass_kernel_spmd(nc, in_maps, list(range(N_CORES)))
    LAST_RESULTS = res
    out = np.concatenate(
        [res.results[c]["out"][:nd_core] for c in range(N_CORES)], axis=0)
    return out.astype(np.float32)



# revision 4
# speedup vs baseline: 1.1972x; 1.1972x over previous
"""Trainium2 Bass kernel: single-head GATConv (+ self-loops, segment softmax)
followed by LayerNorm, distributed over 8 NeuronCores.

Distribution (destination-sharded SPMD):
  * Host: append self-loops, sort edges by destination, shard destinations
    contiguously across cores.  Within a core, destinations form 128-wide
    blocks.  Each block's edges are split into 8 subgroups:
    (sign of a_s[src]+a_d[dst]) x (src bank of 25088 rows, for int16
    dma_gather indices), each padded to a multiple of 128 slots.  Subgroup
    widths are uniform across blocks/cores so one program serves all cores.
  * Softmax factorization: exp(leaky_relu(s+d)) = max(us*ud, vs*vd) with
    us=exp(s), vs=exp(0.2 s) (per-source), ud=exp(d), vd=exp(0.2 d)
    (per-dest).  The host decides each edge's max-branch by sign (control
    only); the device then needs NO per-edge transcendentals: the dest
    factors are uniform per matmul column group and pull out of the
    aggregation matmul.  The denominator is skipped when bias==0: LayerNorm
    is invariant to a positive per-node scale.
  * Device phase A (replicated): hext[n] = bf16 row
    [h(0:64) | 1 | a_s | a_d | pad] (128 cols = 256 B, dma_gather's
    minimum row size).
  * Device phase B: per chunk of CB blocks: dma_gather fetches hext[src]
    rows into G (bf16); U = exp(a_s), V = exp(0.2 a_s) (batched ACT); per
    column: one fused DVE op builds A^T = (iota == dst_rel) * U_col (all
    bf16), one bf16 matmul accumulates into the pos/neg halves of the
    block's PSUM acc; per-block epilogue applies ud/vd column scales and
    LayerNorm (rsqrt via exp(-0.5 ln(var+eps)) so ACT stays on one
    function table).
"""

import math

import numpy as np
import ml_dtypes

import concourse.bacc as bacc
import concourse.bass as bass
import concourse.tile as tile
from concourse import mybir
from concourse.bass_utils import run_bass_kernel_spmd

P = 128
HEXT_W = 128          # bf16 row = 256 B (dma_gather needs 256B-multiple rows)
COL_ONES = 64
COL_AS = 65
COL_AD = 66
N_BANKS = 4           # dma_gather int16 indices: banks of N_pad//4 <= 32767
N_GROUPS = 2          # positive / negative leaky branch
GCOLS = 8             # gather-call column granularity (1024 descs/ring)

f32 = mybir.dt.float32
bf16 = mybir.dt.bfloat16
i16 = mybir.dt.int16

LEAK = 0.2
LN_EPS = 1e-5

bfdt = ml_dtypes.bfloat16


def _cdiv(a, b):
    return -(-a // b)


# ---------------------------------------------------------------------------
# Host-side preprocessing
# ---------------------------------------------------------------------------

def prep_edges(x, edge_index, W, att_src, att_dst, n_cores, bank_size):
    """Shard + sort edges, split into (sign x bank) subgroups per 128-dest
    block, pad subgroups to x128 slots with uniform widths.

    Returns (per_core, S_gk, NB, nd_core):
      per_core[c] = dict(dr=..., src_slot=...) where
        dr:  [P, TC] per-slot dest-slot-in-block (-1 pad)
        src_slot: [P, TC] bank-local hext row per slot
      S_gk: [8] list of per-subgroup column counts (order: g-major, k-minor)
    """
    N, D = x.shape
    assert N % n_cores == 0
    nd_core = N // n_cores
    NB = _cdiv(nd_core, P)

    ws = (W @ att_src).astype(np.float64)
    wd = (W @ att_dst).astype(np.float64)
    a_s = (x.astype(np.float64) @ ws).astype(np.float32)
    a_d = (x.astype(np.float64) @ wd).astype(np.float32)

    src = np.asarray(edge_index[0]).astype(np.int64)
    dst = np.asarray(edge_index[1]).astype(np.int64)
    loops = np.arange(N, dtype=np.int64)
    src = np.concatenate([src, loops])
    dst = np.concatenate([dst, loops])

    order = np.argsort(dst, kind="stable")
    s_dst = dst[order]
    s_src = src[order]
    sign = (a_s[s_src] + a_d[s_dst]) <= 0.0   # False=pos group0, True=neg

    bounds = np.searchsorted(s_dst, np.arange(0, N + nd_core, nd_core))

    # per (core, block, group, bank) counts.  Each core's hext uses node
    # order rotated by c*nd_core so its own destinations sit at rows
    # [0, nd_core) — the SPMD program then loads dest rows at fixed
    # addresses on every core.
    cnts = np.zeros((n_cores, NB, N_GROUPS, N_BANKS), dtype=np.int64)
    per_core_raw = []
    for c in range(n_cores):
        lo, hi = int(bounds[c]), int(bounds[c + 1])
        d_loc = s_dst[lo:hi] - c * nd_core
        blk = d_loc >> 7
        g = sign[lo:hi].astype(np.int64)
        src_row = (s_src[lo:hi] - c * nd_core) % N   # rotated hext row
        k = src_row // bank_size
        key = ((blk * N_GROUPS + g) * N_BANKS + k)
        cnts[c] += np.bincount(key, minlength=NB * 8).reshape(NB, 2, 4)
        per_core_raw.append((d_loc, src_row, blk, g, k, key))

    S_gk = [int(_cdiv(int(cnts[:, :, g, k].max()), P))
            for g in range(N_GROUPS) for k in range(N_BANKS)]
    TC_BLK = sum(S_gk)

    off_gk = np.concatenate([[0], np.cumsum(S_gk)])[:-1]

    per_core = []
    for c in range(n_cores):
        d_loc, src_c, blk, g, k, key = per_core_raw[c]
        order2 = np.argsort(key, kind="stable")
        d2 = d_loc[order2]
        s2 = src_c[order2]
        key2 = key[order2]
        starts = np.zeros(NB * 8 + 1, dtype=np.int64)
        starts[1:] = np.cumsum(np.bincount(key2, minlength=NB * 8))
        pos_in = np.arange(len(key2)) - starts[key2]
        b2 = key2 // 8
        gk2 = key2 % 8
        s_col = pos_in >> 7
        lane = pos_in & 127
        col = b2 * TC_BLK + off_gk[gk2] + s_col
        TC = NB * TC_BLK
        dr = np.full((P, TC), -1.0, dtype=np.float32)
        src_slot = np.zeros((P, TC), dtype=np.int64)
        dr[lane, col] = (d2 & 127).astype(np.float32)
        src_slot[lane, col] = s2 % bank_size  # bank-local row (pads stay 0)
        per_core.append({"dr": dr, "src_slot": src_slot})
    return per_core, S_gk, NB, nd_core, TC_BLK


def build_idx_slabs(per_core, S_gk, NB, CB, TC_BLK):
    """Per-core int16 idx slab [P, n_chunks * chunk_idx_cols], where each
    chunk's region is the concatenation of its 8 calls' packed index arrays.
    Call (gk) covers columns off_gk..off_gk+CB*S_gk-1 (block-major within
    call) of the chunk; idx i of a call = slot (i%128, col i//128), packed
    int16: position i -> partition i%16, word i//16, replicated across the
    8 gpsimd core groups."""
    n_chunks = NB // CB
    off_gk = np.concatenate([[0], np.cumsum(S_gk)])[:-1]
    out = []
    for data in per_core:
        src_slot = data["src_slot"]  # [P, NB*TC_BLK] bank-local rows
        slabs = []
        for ch in range(n_chunks):
            for gk in range(8):
                sgk = S_gk[gk]
                cols = []
                for b in range(ch * CB, (ch + 1) * CB):
                    c0 = b * TC_BLK + off_gk[gk]
                    cols.append(src_slot[:, c0:c0 + sgk])
                call = np.concatenate(cols, axis=1)  # [P, CB*sgk]
                n = call.shape[1] * P
                flat = call.T.reshape(-1)  # i = col*128+lane
                packed = np.zeros((16, n // 16), dtype=np.int16)
                packed[np.arange(n) % 16, np.arange(n) // 16] = (
                    flat.astype(np.uint16).view(np.int16))
                slabs.append(np.tile(packed, (8, 1)))
        out.append(np.concatenate(slabs, axis=1))
    return out


def build_dr_slab(per_core, S_gk, NB, CB, TC_BLK):
    """Rearrange dr into per-chunk call-major column order used on device:
    chunk columns = [gk0: blocks b0..b0+CB x S_0 | gk1: ... ] — bf16."""
    n_chunks = NB // CB
    off_gk = np.concatenate([[0], np.cumsum(S_gk)])[:-1]
    out = []
    for data in per_core:
        dr = data["dr"]
        pieces = []
        for ch in range(n_chunks):
            for gk in range(8):
                sgk = S_gk[gk]
                for b in range(ch * CB, (ch + 1) * CB):
                    c0 = b * TC_BLK + off_gk[gk]
                    pieces.append(dr[:, c0:c0 + sgk])
        out.append(np.ascontiguousarray(np.concatenate(pieces, axis=1)))
    return out


def prep_xT(x, sup):
    """Pad x, permute within super-tiles so contiguous lhsT slices produce
    per-partition-contiguous hext stores, return [D, N_pad] bf16."""
    N, D = x.shape
    n_sup = _cdiv(N, sup)
    N_pad = n_sup * sup
    xpad = np.zeros((N_pad, D), dtype=np.float32)
    xpad[:N] = x
    MP = sup // P
    xr = xpad.reshape(n_sup, P, MP, D)
    xperm = xr.transpose(0, 2, 1, 3).reshape(N_pad, D)
    return np.ascontiguousarray(xperm.T).astype(bfdt), N_pad


def make_w_aug(W, att_src, att_dst):
    D = W.shape[0]
    w_aug = np.zeros((D, 66), dtype=np.float32)
    w_aug[:, :D] = W
    w_aug[:, 64] = W @ att_src
    w_aug[:, 65] = W @ att_dst
    return w_aug


# ---------------------------------------------------------------------------
# Device program
# ---------------------------------------------------------------------------

def build_program(N_pad, D, NB, S_gk, CB, SUP, w_aug, general,
                  ln_bias=None, ln_gamma=None, ln_beta=None,
                  debug_stage=None, n_queues=4):
    assert NB % CB == 0
    assert SUP % P == 0 and N_pad % SUP == 0
    assert N_pad % N_BANKS == 0
    bank_size = N_pad // N_BANKS
    assert bank_size <= 32768
    MP = SUP // P
    n_sup = N_pad // SUP
    n_chunks = NB // CB
    TC_BLK = sum(S_gk)
    CS = CB * TC_BLK                  # columns per chunk
    NCOL = 65 if general else 64      # matmul rhs width
    ACC_W = 2 * NCOL                  # pos half | neg half
    off_gk = [0]
    for s in S_gk[:-1]:
        off_gk.append(off_gk[-1] + s)
    IDX_COLS = CS * 8                 # int16 words per chunk ( CS*128/16 )

    nc = bacc.Bacc(num_swdge_queues=n_queues)
    xT_d = nc.declare_dram_parameter("xT", [D, N_pad], bf16, isOutput=False)
    idx_d = nc.declare_dram_parameter(
        "idx", [P, n_chunks * IDX_COLS], i16, isOutput=False)
    dr_d = nc.declare_dram_parameter(
        "dr", [P, n_chunks * CS], f32, isOutput=False)
    out_d = nc.declare_dram_parameter("out", [NB * P, D], f32, isOutput=True)
    hext = nc.dram_tensor("hext", [N_pad, HEXT_W], bf16)

    w_aug_t = nc.inline_tensor(w_aug.astype(bfdt), "w_aug")
    iota_np = np.broadcast_to(
        np.arange(P, dtype=np.float32), (P, P)).astype(bfdt).copy()
    iota_t = nc.inline_tensor(iota_np, "iota_rows")
    if general:
        def _rep(v):
            return np.ascontiguousarray(np.broadcast_to(
                np.asarray(v, dtype=np.float32).reshape(1, D), (P, D)))
        bias_t = nc.inline_tensor(_rep(ln_bias), "ln_bias")
        gamma_t = nc.inline_tensor(_rep(ln_gamma), "ln_gamma")
        beta_t = nc.inline_tensor(_rep(ln_beta), "ln_beta")

    with tile.TileContext(nc) as tc:
        with tc.tile_pool(name="const", bufs=1) as cpool:
            iota_sb = cpool.tile([P, P], bf16, tag="c_iota")
            nc.sync.dma_start(out=iota_sb[:], in_=iota_t[:])
            waug_sb = cpool.tile([D, 66], bf16, tag="c_waug")
            nc.sync.dma_start(out=waug_sb[:], in_=w_aug_t[:])
            eps_sb = cpool.tile([P, 1], f32, tag="c_eps")
            nc.vector.memset(eps_sb[:], LN_EPS)
            if general:
                bias_sb = cpool.tile([P, D], f32, tag="c_bias")
                nc.sync.dma_start(out=bias_sb[:], in_=bias_t[:])
                gamma_sb = cpool.tile([P, D], f32, tag="c_gamma")
                nc.sync.dma_start(out=gamma_sb[:], in_=gamma_t[:])
                beta_sb = cpool.tile([P, D], f32, tag="c_beta")
                nc.sync.dma_start(out=beta_sb[:], in_=beta_t[:])

            # ---------------- Phase A ------------------------------------
            with tc.tile_pool(name="pa_x", bufs=3) as pa_x, \
                 tc.tile_pool(name="pa_ps", bufs=4, space="PSUM") as pa_ps, \
                 tc.tile_pool(name="pa_h", bufs=3) as pa_h:
                for gsup in range(n_sup):
                    xt_sb = pa_x.tile([D, SUP], bf16)
                    nc.sync.dma_start(
                        out=xt_sb[:], in_=xT_d[:, gsup * SUP:(gsup + 1) * SUP])
                    hx_sb = pa_h.tile([P, MP, HEXT_W], bf16)
                    for q in range(MP // 4):
                        ps = pa_ps.tile([P, 4, 66], f32)
                        for kk in range(4):
                            m = q * 4 + kk
                            nc.tensor.matmul(
                                ps[:, kk, :],
                                lhsT=xt_sb[:, m * P:(m + 1) * P],
                                rhs=waug_sb[:],
                                start=True, stop=True,
                            )
                        sl = slice(q * 4, (q + 1) * 4)
                        if q % 2 == 0:
                            nc.vector.tensor_copy(
                                out=hx_sb[:, sl, 0:64], in_=ps[:, :, 0:64])
                        else:
                            nc.scalar.copy(
                                out=hx_sb[:, sl, 0:64], in_=ps[:, :, 0:64])
                        nc.vector.tensor_copy(
                            out=hx_sb[:, sl, COL_AS:COL_AD + 1],
                            in_=ps[:, :, 64:66])
                    nc.vector.memset(hx_sb[:, :, COL_ONES:COL_ONES + 1], 1.0)
                    nc.vector.memset(hx_sb[:, :, COL_AD + 1:], 0.0)
                    nc.sync.dma_start(
                        out=hext[gsup * SUP:(gsup + 1) * SUP, :].rearrange(
                            "(p m) c -> p m c", m=MP),
                        in_=hx_sb[:],
                    )

            # ---------------- Phase B ------------------------------------
            with tc.tile_pool(name="pb_io", bufs=2) as pb_io, \
                 tc.tile_pool(name="pb_g", bufs=2) as pb_g, \
                 tc.tile_pool(name="pb_uv", bufs=2) as pb_uv, \
                 tc.tile_pool(name="pb_at", bufs=4) as pb_at, \
                 tc.tile_pool(name="pb_y", bufs=3) as pb_y, \
                 tc.tile_pool(name="pb_sm", bufs=6) as pb_sm, \
                 tc.tile_pool(name="pb_ps", bufs=4, space="PSUM") as pb_ps:
                for ch in range(n_chunks if debug_stage != "A" else 0):
                    idx_sb = pb_io.tile([P, IDX_COLS], i16)
                    nc.sync.dma_start(
                        out=idx_sb[:],
                        in_=idx_d[:, ch * IDX_COLS:(ch + 1) * IDX_COLS])
                    dr_sb = pb_io.tile([P, CS], f32)
                    nc.sync.dma_start(
                        out=dr_sb[:], in_=dr_d[:, ch * CS:(ch + 1) * CS])
                    hb_sb = pb_io.tile([P, CB, HEXT_W], bf16)
                    nc.sync.dma_start(
                        out=hb_sb[:],
                        in_=hext[ch * CB * P:(ch + 1) * CB * P, :].rearrange(
                            "(b p) c -> p b c", p=P))

                    G = pb_g.tile([P, CS, HEXT_W], bf16)
                    iw = 0  # idx word offset within the chunk slab
                    qn = 0
                    for gk in range(8):
                        sgk = S_gk[gk]
                        ncols = CB * sgk
                        kbank = gk % N_BANKS
                        for c0 in range(0, ncols, GCOLS):
                            cols = min(GCOLS, ncols - c0)
                            nidx = cols * P
                            nc.gpsimd.dma_gather(
                                out_ap=G[:, CB * off_gk[gk] + c0:
                                         CB * off_gk[gk] + c0 + cols, :],
                                in_ap=hext[kbank * bank_size:
                                           (kbank + 1) * bank_size, :],
                                idxs_ap=idx_sb[:, iw:iw + nidx // 16],
                                num_idxs=nidx, num_idxs_reg=nidx,
                                elem_size=HEXT_W,
                                queue_num=qn)
                            qn = (qn + 1) % n_queues
                            iw += nidx // 16
                    if debug_stage == "gather":
                        continue

                    # U/V per-source factors, batched ACT (bf16)
                    u_t = pb_uv.tile([P, CS], f32)
                    v_t = pb_uv.tile([P, CS], f32)
                    nc.scalar.activation(
                        out=u_t[:], in_=G[:, :, COL_AS],
                        func=mybir.ActivationFunctionType.Exp)
                    nc.scalar.activation(
                        out=v_t[:], in_=G[:, :, COL_AS],
                        func=mybir.ActivationFunctionType.Exp, scale=LEAK)

                    # dest factors ud/vd per block
                    ud_t = pb_uv.tile([P, CB], f32)
                    vd_t = pb_uv.tile([P, CB], f32)
                    nc.scalar.activation(
                        out=ud_t[:], in_=hb_sb[:, :, COL_AD],
                        func=mybir.ActivationFunctionType.Exp)
                    nc.scalar.activation(
                        out=vd_t[:], in_=hb_sb[:, :, COL_AD],
                        func=mybir.ActivationFunctionType.Exp, scale=LEAK)
                    if debug_stage == "uv":
                        continue

                    for bb in range(CB):
                        b = ch * CB + bb
                        acc = pb_ps.tile([P, ACC_W], f32)
                        for g in range(N_GROUPS):
                            scal = u_t if g == 0 else v_t
                            half = slice(0, NCOL) if g == 0 else \
                                slice(NCOL, 2 * NCOL)
                            colss = []
                            for k in range(N_BANKS):
                                gk = g * N_BANKS + k
                                c0 = CB * off_gk[gk] + bb * S_gk[gk]
                                colss.extend(range(c0, c0 + S_gk[gk]))
                            for ii, cc in enumerate(colss):
                                at = pb_at.tile([P, P], bf16)
                                nc.vector.tensor_scalar(
                                    out=at[:], in0=iota_sb[:],
                                    scalar1=dr_sb[:, cc:cc + 1],
                                    scalar2=scal[:, cc:cc + 1],
                                    op0=mybir.AluOpType.is_equal,
                                    op1=mybir.AluOpType.mult,
                                )
                                nc.tensor.matmul(
                                    acc[:, half], lhsT=at[:],
                                    rhs=G[:, cc, 0:NCOL],
                                    start=(ii == 0),
                                    stop=(ii == len(colss) - 1),
                                )
                        # epilogue
                        t1 = pb_y.tile([P, D], f32)
                        nc.scalar.activation(
                            out=t1[:], in_=acc[:, 0:D],
                            func=mybir.ActivationFunctionType.Copy,
                            scale=ud_t[:, bb:bb + 1])
                        t2 = pb_y.tile([P, D], f32)
                        nc.scalar.activation(
                            out=t2[:], in_=acc[:, NCOL:NCOL + D],
                            func=mybir.ActivationFunctionType.Copy,
                            scale=vd_t[:, bb:bb + 1])
                        y0 = pb_y.tile([P, D], f32)
                        nc.vector.tensor_add(out=y0[:], in0=t1[:], in1=t2[:])
                        if general:
                            den = pb_sm.tile([P, 1], f32)
                            nc.scalar.activation(
                                out=den[:], in_=acc[:, 64:65],
                                func=mybir.ActivationFunctionType.Copy,
                                scale=ud_t[:, bb:bb + 1])
                            den2 = pb_sm.tile([P, 1], f32)
                            nc.scalar.activation(
                                out=den2[:], in_=acc[:, NCOL + 64:NCOL + 65],
                                func=mybir.ActivationFunctionType.Copy,
                                scale=vd_t[:, bb:bb + 1])
                            nc.vector.tensor_add(
                                out=den[:], in0=den[:], in1=den2[:])
                            rec = pb_sm.tile([P, 1], f32)
                            nc.vector.reciprocal(rec[:], den[:])
                            nc.vector.tensor_scalar_mul(
                                out=y0[:], in0=y0[:], scalar1=rec[:])
                            nc.vector.tensor_add(
                                out=y0[:], in0=y0[:], in1=bias_sb[:])
                        st = pb_sm.tile([P, 6], f32)
                        nc.vector.bn_stats(out=st[:], in_=y0[:])
                        mv = pb_sm.tile([P, 2], f32)
                        nc.vector.bn_aggr(out=mv[:], in_=st[:])
                        # rsqrt(var+eps) = exp(-0.5 * ln(var+eps)): keeps ACT
                        # on the ln/exp/copy function table (no reloads).
                        sd = pb_sm.tile([P, 1], f32)
                        nc.scalar.activation(
                            out=sd[:], in_=mv[:, 1:2],
                            func=mybir.ActivationFunctionType.Ln,
                            bias=eps_sb[:])
                        nc.scalar.activation(
                            out=sd[:], in_=sd[:],
                            func=mybir.ActivationFunctionType.Exp,
                            scale=-0.5)
                        y = pb_y.tile([P, D], f32)
                        nc.vector.tensor_scalar(
                            out=y[:], in0=y0[:],
                            scalar1=mv[:, 0:1], scalar2=sd[:],
                            op0=mybir.AluOpType.subtract,
                            op1=mybir.AluOpType.mult,
                        )
                        if general:
                            nc.vector.tensor_mul(
                                out=y[:], in0=y[:], in1=gamma_sb[:])
                            nc.vector.tensor_add(
                                out=y[:], in0=y[:], in1=beta_sb[:])
                        nc.sync.dma_start(
                            out=out_d[b * P:(b + 1) * P, :], in_=y[:])
    nc.finalize()
    return nc


# ---------------------------------------------------------------------------
# Entry point
# ---------------------------------------------------------------------------

N_CORES = 8
SUP_DEFAULT = 2048

LAST_RESULTS = None


def _pick_cb(NB):
    for cb in (7, 8, 6, 5, 4, 2):
        if NB % cb == 0:
            return cb
    return 1


def build_all(x, edge_index, W, att_src, att_dst, bias, gamma, beta):
    """Host prep + device program. Returns (nc, in_maps, nd_core, S_gk)."""
    x = np.asarray(x, dtype=np.float32)
    W = np.asarray(W, dtype=np.float32)
    att_src = np.asarray(att_src, dtype=np.float32)
    att_dst = np.asarray(att_dst, dtype=np.float32)
    bias = np.asarray(bias, dtype=np.float32)
    gamma = np.asarray(gamma, dtype=np.float32)
    beta = np.asarray(beta, dtype=np.float32)
    N, D = x.shape

    _, N_pad = prep_xT(x, SUP_DEFAULT)
    bank_size = N_pad // N_BANKS
    nd_core = N // N_CORES
    xTs = [prep_xT(np.roll(x, -c * nd_core, axis=0), SUP_DEFAULT)[0]
           for c in range(N_CORES)]
    per_core, S_gk, NB, nd_core, TC_BLK = prep_edges(
        x, edge_index, W, att_src, att_dst, N_CORES, bank_size)
    CB = _pick_cb(NB)
    idx_slabs = build_idx_slabs(per_core, S_gk, NB, CB, TC_BLK)
    dr_slabs = build_dr_slab(per_core, S_gk, NB, CB, TC_BLK)
    w_aug = make_w_aug(W, att_src, att_dst)
    general = not (
        np.all(bias == 0.0) and np.all(gamma == 1.0) and np.all(beta == 0.0))

    nc = build_program(
        N_pad, D, NB, S_gk, CB, SUP_DEFAULT, w_aug, general,
        ln_bias=bias, ln_gamma=gamma, ln_beta=beta)

    in_maps = []
    for c in range(N_CORES):
        in_maps.append(
            {"xT": xTs[c], "idx": idx_slabs[c], "dr": dr_slabs[c]})
    return nc, in_maps, nd_core, S_gk


def kernel(x, edge_index, W, att_src, att_dst, bias, gamma, beta):
    global LAST_RESULTS
    nc, in_maps, nd_core, _ = build_all(
        x, edge_index, W, att_src, att_dst, bias, gamma, beta)
    res = run_bass_kernel_spmd(nc, in_maps, list(range(N_CORES)))
    LAST_RESULTS = res
    out = np.concatenate(
        [res.results[c]["out"][:nd_core] for c in range(N_CORES)], axis=0)
    return out.astype(np.float32)
